# revision 16
# baseline (speedup 1.0000x reference)
"""EmbeddingBag-mean (padded ragged gather + masked mean) on 8 Trainium2 cores.

Strategy (data-parallel over batch):
  - Each of the 8 cores owns B/8 = 2048 samples; the embedding table is
    replicated to every core's HBM as fp16, rows padded to 128 elements
    (256 B stride) so the MoE `dma_gather` ucode (stride in 256 B units,
    int16 indices) can gather single 128 B rows.
  - int16 indices only reach 32768 rows, so the gather runs as NQ=4 passes
    over overlapping 32768-row windows whose bases (spacing 25001) tile a
    circle: the first WIN-spacing core rows are duplicated after the table
    end, so every pass has the same (small) exclusive index mass.  A zero
    sentinel row sits at each window base (relative index 0).  Flexible
    indices (in two windows' overlap) sit on edges of a cycle; per sample,
    exact min-max water-filling (binary search + wrap-edge scan) equalizes
    per-pass counts to ~ceil(len/4).
  - Samples are dealt greedily to 16 global blocks of 1024 (8 cores x 128
    partitions) minimizing the per-block uniform slot count G[b] = max over
    samples of the per-pass max (shared across cores; one SPMD module).
    Every (block, pass) gets exactly G[b] slots; pad slots -> sentinel row.
  - Device kernel (per core): blocks are grouped (NGRP=4 groups of 4
    consecutive blocks).  Per group, ONE dma_gather per pass covers all its
    blocks (sample-major column order: for b, for j<G[b], one column of 128
    samples), so a rep issues only NGRP*NQ = 16 gather instructions -- the
    ~1 us fixed SWDGE cost per instruction made the old one-gather-per-
    (block, pass) structure (80 instructions) the bottleneck, while very
    large gathers (4 instructions/rep) stall the 128-entry descriptor
    ring; 16 is the measured sweet spot.  Per group:
      1. fold the NQ pass regions into one [128, S, 64] fp16 tile with
         contiguous DVE tensor_adds (fast mode)
      2. per block, one DVE tensor_reduce over its G[b] slot columns
         ([p, d, c] strided pattern) -> fp32
      3. ACT Copy-with-scale by 1/max(len,1) (per-partition scalar)
      4. DMA the [128, 64] fp32 block out
  - Host un-permutes the global deal and returns [B, 64] fp32.
"""

import numpy as np

try:
    import concourse.bacc as bacc
except ImportError:  # harness containers keep the repo at /opt/trn_rl_repo
    import sys

    sys.path.insert(0, "/opt/trn_rl_repo")
    import concourse.bacc as bacc

import concourse.bass as bass
import concourse.mybir as mybir
import concourse.tile as tile
from concourse import bass_utils

B, L, V, D = 16384, 50, 100000, 64
NCORES = 8
P = 128
BC = B // NCORES  # 2048 samples per core
NBLK = BC // P  # 16 blocks of 128 samples
NQ = 4  # gather passes (overlapping windows)
WIN = 32768  # int16-reachable rows per pass
DEVC = V + NQ  # core device rows: table + one zero sentinel per window
NGRP = 4  # block groups; NGRP*NQ gather instructions per rep

# wrap layout: windows tile a circle (the first WIN-S core rows are
# duplicated after the end), so the pass-exclusive mass is uniform instead
# of piling onto the outer windows.  Sentinel zero row at each window base.
_SPACING = -(-DEVC // NQ)  # ceil
BASES = [q * _SPACING for q in range(NQ)]
DEVROWS = BASES[-1] + WIN  # core + duplicated prefix
assert DEVROWS >= DEVC and _SPACING < WIN

_CACHE: dict = {}


def _manual_dma_gather(nc, out_ap, in_ap, idxs_ap, num_idxs, elem_size,
                       queue_num, single_packet):
    """dma_gather without the elem_size_bytes%256 assert (stride is 256B)."""
    gp = nc.gpsimd
    _in_ap = gp.lower_ap_dma(in_ap, for_custom_bir_dma=True)
    _idxs_ap = gp.lower_ap(idxs_ap)
    _out_ap = gp.lower_ap(out_ap)
    stride_bytes = in_ap.ap[0][0] * mybir.dt.size(in_ap.dtype)
    assert stride_bytes % 256 == 0
    return gp.add_instruction(
        mybir.InstDMAGatherAnt(
            name=nc.get_next_instruction_name(),
            ins=[*_in_ap, _idxs_ap, gp.lower_val_access(gp.to_reg(num_idxs))],
            outs=[_out_ap],
            transpose=False,
            num_idxs=num_idxs,
            elem_size=elem_size,
            stride_bytes_256=stride_bytes // 256,
            gen_mode=0,
            single_packet=single_packet,
            queue_num=queue_num,
            sbuf_tokens_per_rank=0,
            sbuf_free_dim_per_rank=0,
            sbuf_free_dim_pad_per_rank=0,
            sbuf_byte_offset=0,
        )
    )


GROUP_MODE = "inter"  # "consec" | "inter" (balanced gather sizes)


def _groups():
    """Partition block ids 0..NBLK-1 into NGRP runs."""
    per = NBLK // NGRP
    if GROUP_MODE == "inter":
        return [list(range(g, NBLK, NGRP)) for g in range(NGRP)]
    return [list(range(g * per, (g + 1) * per)) for g in range(NGRP)]


def build(g_sched, reps: int = 1, mode: str = "full"):
    """Build + compile the per-core Bass module.

    g_sched: [NBLK][NQ] slot counts; uniform per block (G[b] = max of row).
    reps > 1 wraps the block loop in tc.For_i for slope timing.
    mode: "full" | "gather" (skip reduce/scale/out).
    """
    g_sched = [list(r) for r in g_sched]
    assert len(g_sched) == NBLK and all(len(r) == NQ for r in g_sched)
    G = [max(1, max(r)) for r in g_sched]
    groups = _groups()
    S = [sum(G[b] for b in grp) for grp in groups]  # columns per pass region
    # idx16 column layout: per (group, pass) a run of 128*S[g]/16 int16 cols
    WC = sum(NQ * 8 * s for s in S)

    nc = bacc.Bacc("TRN2", target_bir_lowering=False, debug=False,
                   num_swdge_queues=4)
    table = nc.dram_tensor("table", [DEVROWS, P], mybir.dt.float16,
                           kind="ExternalInput")
    idx = nc.dram_tensor("idx", [P, WC], mybir.dt.int16, kind="ExternalInput")
    inv_len = nc.dram_tensor("inv_len", [P, NBLK], mybir.dt.float32,
                             kind="ExternalInput")
    out = nc.dram_tensor("out", [NBLK, P, D], mybir.dt.float32,
                         kind="ExternalOutput")

    with tile.TileContext(nc) as tc:
        with (
            tc.tile_pool(name="const", bufs=1) as cpool,
            tc.tile_pool(name="gather", bufs=2) as gpool,
            tc.tile_pool(name="fold", bufs=1) as fpool,
            tc.tile_pool(name="res", bufs=4) as rpool,
        ):
            idx_sb = cpool.tile([P, WC], mybir.dt.int16)
            nc.sync.dma_start(idx_sb[:], idx.ap())
            invl_sb = cpool.tile([P, NBLK], mybir.dt.float32)
            nc.sync.dma_start(invl_sb[:], inv_len.ap())

            # NGRP*NQ gathers per rep; queue pattern must repeat with period
            # dividing 8 (Tile's DMASW lanes lock to their first user's
            # queue); round-robin keeps the 4 SWDGE queues evenly loaded.
            ictr = [0]

            def body():
                col = 0
                for gi, grp in enumerate(groups):
                    sg = S[gi]
                    g = gpool.tile([P, NQ * sg, D], mybir.dt.float16,
                                   tag=f"g{gi}")
                    for q in range(NQ):
                        win = table.ap()[BASES[q] : BASES[q] + WIN, :D]
                        _manual_dma_gather(
                            nc,
                            g[:, q * sg : (q + 1) * sg, :],
                            win,
                            idx_sb[:, col : col + 8 * sg],
                            sg * P,
                            D,
                            queue_num=ictr[0] % 4,
                            single_packet=False,
                        )
                        ictr[0] += 1
                        col += 8 * sg
                    if mode == "gather":
                        continue
                    # stage 1: fold the NQ pass regions into one [P, sg, D]
                    # fp16 tile with contiguous tensor_adds (DVE fast mode);
                    # stage 2: per-block strided reduce over its G[b] slots.
                    ta = fpool.tile([P, sg, D], mybir.dt.float16,
                                    tag=f"ta{gi}")
                    tb = fpool.tile([P, sg, D], mybir.dt.float16,
                                    tag=f"tb{gi}")
                    nc.vector.tensor_add(
                        ta[:], g[:, 0:sg, :], g[:, sg : 2 * sg, :])
                    src = ta
                    for q in range(2, NQ):
                        dst = tb if (q % 2 == 0) else fpool.tile(
                            [P, sg, D], mybir.dt.float16, tag=f"ta{gi}")
                        nc.vector.tensor_add(
                            dst[:], src[:], g[:, q * sg : (q + 1) * sg, :])
                        src = dst
                    fv = src[:, :, :].rearrange("p c d -> p d c")
                    off = 0
                    for b in grp:
                        rt = rpool.tile([P, D], mybir.dt.float32, tag="red")
                        nc.vector.tensor_reduce(
                            out=rt[:],
                            in_=fv[:, :, off : off + G[b]],
                            axis=mybir.AxisListType.X,
                            op=mybir.AluOpType.add,
                        )
                        off += G[b]
                        o = rpool.tile([P, D], mybir.dt.float32, tag="o")
                        nc.scalar.activation(
                            o[:],
                            rt[:],
                            mybir.ActivationFunctionType.Copy,
                            scale=invl_sb[:, b : b + 1],
                        )
                        nc.sync.dma_start(out.ap()[b], o[:])

            if reps == 1:
                body()
            else:
                with tc.For_i(0, reps, 1):
                    body()

    nc.compile()
    return nc


def _dev_table(table):
    """fp16 device table [DEVROWS, 128]: zero sentinel at each window base,
    original row r at core position devpos[r], first DEVROWS-DEVC core rows
    duplicated after the end (wrap)."""
    t16 = np.asarray(table, dtype=np.float32).astype(np.float16)
    dev = np.zeros((DEVROWS, P), np.float16)
    devpos = np.empty(V, np.int64)
    src = 0
    bset = set(BASES)
    for pos in range(DEVC):
        if pos in bset:
            continue  # zero sentinel
        dev[pos, :D] = t16[src]
        devpos[src] = pos
        src += 1
    assert src == V
    dev[DEVC:] = dev[: DEVROWS - DEVC]
    return dev, devpos


def _feasible_rels(d):
    """[(pass, window-relative idx)] for core position d, incl. wrap copy."""
    out = [(q, d - BASES[q]) for q in range(NQ)
           if BASES[q] <= d < BASES[q] + WIN]
    if d + DEVC < DEVROWS:  # duplicated prefix: reachable from the last pass
        out.append((NQ - 1, d + DEVC - BASES[NQ - 1]))
    return out


def _balance_passes(devrows_sample):
    """Assign each device-row index to a feasible pass, minimizing the max
    per-pass count. Windows overlap adjacently on a circle, so flexible
    indices sit on edges of a cycle -> binary search on T; for each T scan
    the wrap-edge split and run left-greedy water-filling on the path.
    Returns list of NQ lists of window-relative indices."""
    fixed = [[] for _ in range(NQ)]
    flex = [[] for _ in range(NQ)]  # edge e: passes (e, (e+1)%NQ)
    for d in devrows_sample:
        feas = _feasible_rels(d)
        if len(feas) == 1:
            fixed[feas[0][0]].append(feas[0][1])
        else:
            qs = sorted(q for q, _ in feas)
            e = NQ - 1 if qs == [0, NQ - 1] else qs[0]
            flex[e].append(dict(feas))
    f = [len(g) for g in fixed]
    x = [len(e) for e in flex]

    def path_ok(T, t3):
        # t3 wrap items to pass NQ-1; the rest (x[NQ-1]-t3) preload pass 0
        takes = [0] * (NQ - 1)
        carry = x[NQ - 1] - t3
        for q in range(NQ - 1):
            load = f[q] + carry
            if load > T:
                return None
            takes[q] = min(x[q], T - load)
            carry = x[q] - takes[q]
        if f[NQ - 1] + carry + t3 > T:
            return None
        return takes

    lo = max(1, -(-len(devrows_sample) // NQ))
    hi = max(lo, max(f) + sum(x))
    while lo < hi:
        mid = (lo + hi) // 2
        sol = next((
            (t3, tk) for t3 in range(x[NQ - 1] + 1)
            if (tk := path_ok(mid, t3)) is not None), None)
        if sol is not None:
            hi = mid
        else:
            lo = mid + 1
    T = lo
    t3, takes = next((t3, tk) for t3 in range(x[NQ - 1] + 1)
                     if (tk := path_ok(T, t3)) is not None)

    groups = [list(fixed[q]) for q in range(NQ)]
    # wrap edge: t3 items to pass NQ-1, rest to pass 0
    for i, item in enumerate(flex[NQ - 1]):
        q = NQ - 1 if i < t3 else 0
        groups[q].append(item[q])
    for e in range(NQ - 1):
        for i, item in enumerate(flex[e]):
            q = e if i < takes[e] else e + 1
            groups[q].append(item[q])
    return groups


def preprocess(table, indices, lengths):
    """Host prep. Returns (in_maps, g_sched, order) where order[r] is the
    original sample id at global dealt rank r."""
    dev, devpos = _dev_table(table)

    idx_np = np.asarray(indices, dtype=np.int64)  # [B, L]
    lens = np.asarray(lengths).astype(np.int64)  # [B]
    inv_len = (1.0 / np.maximum(lens, 1)).astype(np.float32)

    # per-sample pass groups (window-relative indices)
    sample_groups = []
    cnt = np.zeros((B, NQ), np.int64)
    for s in range(B):
        drows = devpos[idx_np[s, : lens[s]]]
        sample_groups.append(_balance_passes(drows))
        cnt[s] = [len(g) for g in sample_groups[s]]
    cmax = cnt.max(1)

    # greedy deal: assign samples to the 16 global blocks (1024 each) to
    # minimize the per-block slot maxima G[b]; rank r -> (block r//1024,
    # core (r%1024)//128, partition r%128)
    key = cmax * 64 + lens
    pool = np.argsort(-key, kind="stable")
    gmax = np.zeros(NBLK, np.int64)
    fill = np.zeros(NBLK, np.int64)
    assign = np.empty(B, np.int64)
    for s in pool:
        best, bc = -1, None
        for b in range(NBLK):
            if fill[b] >= 1024:
                continue
            cost = max(gmax[b], cmax[s]) - gmax[b]
            if bc is None or cost < bc:
                best, bc = b, cost
        assign[s] = best
        gmax[best] = max(gmax[best], cmax[s])
        fill[best] += 1
    order = np.concatenate([pool[assign[pool] == b] for b in range(NBLK)])

    g_sched = [[int(max(gmax[b], 1))] * NQ for b in range(NBLK)]
    G = [max(1, max(r)) for r in g_sched]
    groups = _groups()
    S = [sum(G[b] for b in grp) for grp in groups]
    WC = sum(NQ * 8 * s for s in S)

    in_maps = []
    for c in range(NCORES):
        idx16 = np.zeros((P, WC), np.int16)
        invl_dev = np.empty((P, NBLK), np.float32)
        col = 0
        for gi, grp in enumerate(groups):
            sg = S[gi]
            for q in range(NQ):
                blk = np.zeros((P, sg), np.int16)  # sentinel rel idx 0
                coff = 0
                for b in grp:
                    ranks = order[b * 1024 + c * P : b * 1024 + (c + 1) * P]
                    if q == 0:
                        invl_dev[:, b] = inv_len[ranks]
                    for p, s in enumerate(ranks):
                        grpq = sample_groups[s][q]
                        blk[p, coff : coff + len(grpq)] = grpq
                    coff += G[b]
                # stream order i = col*128 + p -> wrap int16 [16, n/16] x8
                flat = blk.T.ravel()  # [sg*128]
                w = flat.reshape(sg * P // 16, 16).T  # [16, cols]
                idx16[:, col : col + 8 * sg] = np.tile(w, (8, 1))
                col += 8 * sg
        in_maps.append(
            {
                "table": dev,
                "idx": np.ascontiguousarray(idx16),
                "inv_len": np.ascontiguousarray(invl_dev),
            }
        )
    return in_maps, g_sched, order


def kernel(table, indices, lengths):
    in_maps, g_sched, order = preprocess(table, indices, lengths)
    key = tuple(tuple(r) for r in g_sched)
    nc = _CACHE.get(key)
    if nc is None:
        nc = _CACHE[key] = build(g_sched)
    res = bass_utils.run_bass_kernel_spmd(nc, in_maps, core_ids=list(range(NCORES)))
    full = np.empty((B, D), np.float32)
    for b in range(NBLK):
        for c in range(NCORES):
            ranks = order[b * 1024 + c * P : b * 1024 + (c + 1) * P]
            full[ranks] = res.results[c]["out"][b]
    return full
